# revision 26
# baseline (speedup 1.0000x reference)
"""Trainium2 Bass kernel for DihedralToCartesian (NeRF-style dihedral->xyz chain).

Full-input contract: kernel(angles[65536,252], prev_three[65536,3,3]) -> [65536,126,3].
Batch sharded 8 ways (8192 rows/core = [128 partitions x 64 cols], pure data
parallelism).

Math: the reference renormalizes its frame every step, so the frame follows the
PURE-UNIT dihedral direction (c,s)/sqrt(s^2+c^2) exactly; the 1e-8 damping only
shrinks one step's displacement by |damped| (~5e-7 off except ~4 dataset rows,
worst once-off ~1e-3 rel).  We therefore treat each step as an exact rotation
(no invw/invg normalizer chain):

    h = cd*f2 - sd*f3; u = ca*f1 + sa*h; p += bond*u
    f1' = -u; f2' = sa*f1 - ca*h; f3' = sd*f2 + cd*f3

Sign-folded states g1 = (-1)^(i+1) f1, g2h = (-1)^i f2, m3 = -(-1)^i f3 make
every step sign-free except the bond constant:
    hh  = cd*g2h + sd*m3         g1' = ca*g1 - sa*hh     g2h' = sa*g1 + ca*hh
    m3' = sd*g2h - cd*m3         p' = p + (+/-bond)*g1'
which is 6 plain TT multiplies/adds + 3 STT + 2 ACT const-scales per atom.

Everything runs in fp16 (validated vs the fp32 reference: rel err 9.5e-3 vs the
2e-2 gate): DVE gets its 2x 16-bit mode, SBUF port pressure and DMA bytes halve.
The rsqrt normalization chain stays per-op-rounded fp16 but with fp32-safe
structure.  Planes are stored ATOM-MAJOR so all per-atom loop operands are
contiguous.  Precompute is paced one sub-op per 2 atoms in chunk PAIRS with
activation phases batched (Ln,Ln,Exp,Exp) so table loads stay ~8 total.
Positions accumulate into a dense atom-major pblock; paced transpose pieces
copy it to a j-major staging buffer which DMAs out in ONE 128-descriptor
transfer (64 contiguous DRAM rows per partition).
"""

import json
import os
import sys

import numpy as np

for _p in ("/opt/trn_rl_repo", os.path.expanduser("~/.axon_site/_ro/trn_rl_repo")):
    if os.path.isdir(_p) and _p not in sys.path:
        sys.path.insert(0, _p)

import concourse.bass as bass
import concourse.bacc as bacc
import concourse.mybir as mybir
import concourse.tile as tile
from concourse.bass_utils import run_bass_kernel_spmd

F32 = mybir.dt.float32
F16 = mybir.dt.float16
AOP = mybir.AluOpType
AF = mybir.ActivationFunctionType

N_CORES = 8
B_FULL = 65536
BS = B_FULL // N_CORES
N = 126
P = 128
J = BS // P            # 64
CH = 18                # atoms per precompute chunk
GRP = 2                # chunks per pacing group (phase-batched)
TP = 6                 # atoms per transpose piece (N % TP == 0)

_ALPHA = np.array([2.028, 2.124, 1.941], np.float64)
_BOND = np.array([1.329, 1.458, 1.523], np.float64)
_CA = np.cos(_ALPHA)
_SA = np.sin(_ALPHA)

# engine assignment, overridable: KERN_ENG='{"t3":"v",...}'
# NOTE: scalar_tensor_tensor does NOT compile on Pool ("g") here -- STT ops
# (g1n/g2n/pn) must stay on "v".
ENG = {
    "t1": "v", "t2": "v", "hh": "v", "g1n": "v", "g2n": "v", "m3n": "v",
    "t4": "g", "t3": "g", "pn": "v",
    "fc": "s", "fs": "s",
    "tp0": "s", "tp1": "g",  # transpose piece engine, alternating
    # chunk sub-ops
    "sT": "s", "cT": "s", "s2": "s", "c2": "s", "lg": "s", "rv": "s",
    "nrv": "s", "ss": "g", "cdm": "g", "sdm": "v", "ncdm": "v",
}
ENG.update(json.loads(os.environ.get("KERN_ENG", "{}")))
DT32 = os.environ.get("DT32", "0") == "1"
DT = F32 if DT32 else F16
NPDT = np.float32 if DT32 else np.float16


def _emit(nc: bass.Bass):
    angles = nc.dram_tensor("angles", [BS, 2 * N], DT, kind="ExternalInput").ap()
    prev = nc.dram_tensor("prev_three", [BS, 3, 3], F32, kind="ExternalInput").ap()
    out = nc.dram_tensor("out", [BS, N, 3], DT, kind="ExternalOutput").ap()

    ang_flat = angles.rearrange("(p j) c -> p (j c)", p=P)      # [128, 16128]
    prev_flat = prev.rearrange("(p j) r c -> p (j r c)", p=P)   # [128, 576]
    out_flat = out.rearrange("(p j) a c -> p (j a c)", p=P)     # [128, 24192]

    def eng(key):
        return {"v": nc.vector, "g": nc.gpsimd, "s": nc.scalar}[ENG[key]]

    with tile.TileContext(nc) as tc:
        with (
            tc.tile_pool(name="raw", bufs=1) as rawp,
            tc.tile_pool(name="planes", bufs=1) as planesp,
            tc.tile_pool(name="chunk", bufs=1) as chunkp,
            tc.tile_pool(name="state", bufs=2) as statep,
            tc.tile_pool(name="scratch", bufs=2) as scratch,
            tc.tile_pool(name="big", bufs=1) as bigp,
            tc.tile_pool(name="pv", bufs=1) as pvp,
        ):
            raw = rawp.tile([P, J * 2 * N], DT, tag="raw")       # j-major
            cdp = planesp.tile([P, N * J], DT, tag="cdp")        # atom-major
            sdp = planesp.tile([P, N * J], DT, tag="sdp")
            ncdp = planesp.tile([P, N * J], DT, tag="ncdp")      # -cd (m3n as ADD)
            # per-residue STT scalars as fp16 [P,1] APs (2x mode eligibility)
            cst = pvp.tile([P, 12], DT, tag="cst")
            for k in range(3):
                nc.vector.memset(cst[:][:, k : k + 1], float(-_SA[k]))
                nc.vector.memset(cst[:][:, 3 + k : 4 + k], float(_CA[k]))
                nc.vector.memset(cst[:][:, 6 + k : 7 + k], float(_BOND[k]))
                nc.vector.memset(cst[:][:, 9 + k : 10 + k], float(-_BOND[k]))
            # ring of 2*TP atoms of dense p; transpose pieces drain it
            pblock = bigp.tile([P, 2 * TP * 3 * J], DT, tag="pblock")
            stag = bigp.tile([P, J * N * 3], DT, tag="stag")      # j-major p
            pv = pvp.tile([P, J * 9], F32, tag="pv")
            pvt = pvp.tile([P, 9 * J], F32, tag="pvt")
            ln24 = pvp.tile([P, 1], F32, tag="ln24")
            nc.vector.memset(ln24[:], float(np.log(24.0)))

            nc.sync.dma_start(out=pv[:], in_=prev_flat)
            nc.sync.dma_start(out=raw[:], in_=ang_flat)

            raw_aj = raw[:].rearrange("p (j a) -> p a j", a=2 * N)  # strided view
            cd_a = cdp[:].rearrange("p (a j) -> p a j", j=J)
            sd_a = sdp[:].rearrange("p (a j) -> p a j", j=J)
            ncd_a = ncdp[:].rearrange("p (a j) -> p a j", j=J)
            pb_v = pblock[:].rearrange("p (a x) -> p a x", x=3 * J)  # per-atom dense
            stag_r = stag[:].rearrange("p (j x) -> p x j", x=3 * N)

            # ---------- precompute: one chunk pair, phase-batched --------
            def pair_ops(k0):
                """Thunks for chunks k0, k0+1 (atoms [CH*k0, CH*k0+2*CH))."""
                ops = []
                tiles = {}
                for ci, k in enumerate((k0, k0 + 1)):
                    if k * CH >= N:
                        continue
                    a0 = k * CH
                    sl = slice(a0, a0 + CH)
                    csl = slice(N + a0, N + a0 + CH)
                    sfx = "a" if ci == 0 else "b"
                    SH = [P, CH, J]
                    st = chunkp.tile(SH, DT, tag=f"sT{sfx}", name=f"sT{k}")
                    ct = chunkp.tile(SH, DT, tag=f"cT{sfx}", name=f"cT{k}")
                    s2 = chunkp.tile(SH, DT, tag=f"s2{sfx}", name=f"s2{k}")
                    c2 = chunkp.tile(SH, DT, tag=f"c2{sfx}", name=f"c2{k}")
                    ssq = chunkp.tile(SH, DT, tag=f"ss{sfx}", name=f"ss{k}")
                    lg = chunkp.tile(SH, DT, tag=f"s2{sfx}", name=f"lg{k}")
                    rv = chunkp.tile(SH, DT, tag=f"c2{sfx}", name=f"rv{k}")
                    nrv = chunkp.tile(SH, DT, tag=f"ss{sfx}", name=f"nrv{k}")
                    tiles[k] = (st, ct, s2, c2, ssq, lg, rv, nrv, sl, csl)
                # phase-interleaved order: all sT, all cT, ..., so Ln/Exp batch
                for phase in range(11):
                    for k in (k0, k0 + 1):
                        if k not in tiles:
                            continue
                        st, ct, s2, c2, ssq, lg, rv, nrv, sl, csl = tiles[k]
                        # fp16-safe rsqrt: (24s)^2+(24c)^2 keeps tiny dihedrals
                        # out of fp16-subnormal range; Exp bias ln(24) undoes
                        # the scale exactly: exp(-0.5*ln(576*ss)+ln24) = ss^-0.5
                        if phase == 0:
                            ops.append(lambda st=st, sl=sl:
                                       eng("sT").copy(st[:], raw_aj[:, sl, :]))
                        elif phase == 1:
                            ops.append(lambda ct=ct, csl=csl:
                                       eng("cT").copy(ct[:], raw_aj[:, csl, :]))
                        elif phase == 2:
                            ops.append(lambda s2=s2, st=st:
                                       eng("s2").activation(s2[:], st[:], AF.Square, 0.0, 24.0))
                        elif phase == 3:
                            ops.append(lambda c2=c2, ct=ct:
                                       eng("c2").activation(c2[:], ct[:], AF.Square, 0.0, 24.0))
                        elif phase == 4:
                            ops.append(lambda ssq=ssq, s2=s2, c2=c2:
                                       eng("ss").tensor_add(ssq[:], s2[:], c2[:]))
                        elif phase == 5:
                            ops.append(lambda lg=lg, ssq=ssq:
                                       eng("lg").activation(lg[:], ssq[:], AF.Ln))
                        elif phase == 6:
                            ops.append(lambda rv=rv, lg=lg:
                                       eng("rv").activation(
                                           rv[:], lg[:], AF.Exp, ln24[:], -0.5))
                        elif phase == 7:
                            ops.append(lambda ct=ct, rv=rv, sl=sl:
                                       eng("cdm").tensor_mul(cd_a[:, sl, :], ct[:], rv[:]))
                        elif phase == 8:
                            ops.append(lambda st=st, rv=rv, sl=sl:
                                       eng("sdm").tensor_mul(sd_a[:, sl, :], st[:], rv[:]))
                        elif phase == 9:
                            ops.append(lambda nrv=nrv, rv=rv:
                                       eng("nrv").mul(nrv[:], rv[:], -1.0))
                        else:
                            ops.append(lambda ct=ct, nrv=nrv, sl=sl:
                                       eng("ncdm").tensor_mul(ncd_a[:, sl, :], ct[:], nrv[:]))
                return ops

            # ---------- initial frame from prev_three (fp32) -------------
            nc.scalar.copy(
                pvt[:].rearrange("p (x j) -> p x j", x=9),
                pv[:].rearrange("p (j x) -> p x j", x=9),
            )
            pvt_r = pvt[:].rearrange("p (x j) -> p x j", x=9)
            a0_ap = pvt_r[:, 0:3, :]
            b0_ap = pvt_r[:, 3:6, :]
            c0_ap = pvt_r[:, 6:9, :]

            def rsqrt3(dst, src3, tag):
                sq = scratch.tile([P, 3, J], F32, tag="i_sq", name=f"sq_{tag}")
                nc.scalar.square(sq[:], src3[:])
                s1 = scratch.tile([P, 1, J], F32, tag="i_s1", name=f"s1_{tag}")
                nc.vector.tensor_add(s1[:], sq[:, 0:1, :], sq[:, 1:2, :])
                s2_ = scratch.tile([P, 1, J], F32, tag="i_s2", name=f"s2_{tag}")
                nc.vector.tensor_add(s2_[:], s1[:], sq[:, 2:3, :])
                lgi = scratch.tile([P, 1, J], F32, tag="i_lg", name=f"lg_{tag}")
                nc.scalar.activation(lgi[:], s2_[:], AF.Ln)
                nc.scalar.activation(dst[:], lgi[:], AF.Exp, 0.0, -0.5)

            def cross_into(dst, x, y, eps):
                # dst[c] = (x[c1]*y[c2] + eps) - x[c2]*y[c1]
                for c in range(3):
                    c1, c2 = (c + 1) % 3, (c + 2) % 3
                    m = scratch.tile([P, 1, J], F32, tag="i_cm", name=f"cm{c}_{dst.name}")
                    q = scratch.tile([P, 1, J], F32, tag="i_cq", name=f"cq{c}_{dst.name}")
                    nc.vector.tensor_mul(m[:], x[:, c1 : c1 + 1, :], y[:, c2 : c2 + 1, :])
                    nc.vector.tensor_mul(q[:], x[:, c2 : c2 + 1, :], y[:, c1 : c1 + 1, :])
                    nc.vector.scalar_tensor_tensor(
                        dst[:, c : c + 1, :], m[:], eps, q[:], AOP.add, AOP.subtract)

            vv = scratch.tile([P, 3, J], F32, tag="i_vv")
            nc.vector.scalar_tensor_tensor(
                vv[:], b0_ap, 1e-8, c0_ap, AOP.add, AOP.subtract)
            rv1 = scratch.tile([P, 1, J], F32, tag="i_rv")
            rsqrt3(rv1, vv, "f1")
            f1f = scratch.tile([P, 3, J], F32, tag="i_f1")
            nc.vector.tensor_mul(f1f[:], vv[:], rv1[:].broadcast_to([P, 3, J]))
            g1 = statep.tile([P, 3, J], DT, tag="g1", name="g1_init")
            nc.scalar.copy(g1[:], f1f[:])

            uu = scratch.tile([P, 3, J], F32, tag="i_uu")
            nc.vector.tensor_sub(uu[:], b0_ap, a0_ap)
            # m3_init = +f3 = normalize(cross(b-a, f1) + 1e-8)
            ww = scratch.tile([P, 3, J], F32, tag="i_ww", name="i_ww")
            cross_into(ww, uu, f1f, 1e-8)
            rv2 = scratch.tile([P, 1, J], F32, tag="i_rv", name="i_rv2")
            rsqrt3(rv2, ww, "f3")
            f3f = scratch.tile([P, 3, J], F32, tag="i_f3")
            nc.vector.tensor_mul(f3f[:], ww[:], rv2[:].broadcast_to([P, 3, J]))
            m3 = statep.tile([P, 3, J], DT, tag="m3", name="m3_init")
            nc.scalar.copy(m3[:], f3f[:])
            # g2h_init = -f2 = cross(f1, f3)
            g2f = scratch.tile([P, 3, J], F32, tag="i_g2", name="i_g2")
            cross_into(g2f, f1f, f3f, 0.0)
            g2h = statep.tile([P, 3, J], DT, tag="g2", name="g2_init")
            nc.scalar.copy(g2h[:], g2f[:])
            # p_init = c0 in DT
            p0 = statep.tile([P, 3, J], DT, tag="pd", name="p_init")
            nc.scalar.copy(p0[:], c0_ap)
            p_prev_ap = p0[:]

            # pre-emit first chunk pair; pace the rest at 1 op / 2 atoms
            for f in pair_ops(0):
                f()
            pending = pair_ops(2)
            next_pair = 4

            # ---------- main recurrence ---------------------------------
            for i in range(N):
                k3 = i % 3
                ca, sa = float(_CA[k3]), float(_SA[k3])
                sbond = float(_BOND[k3] * (1.0 if i % 2 == 0 else -1.0))

                cdb = cd_a[:, i : i + 1, :].broadcast_to([P, 3, J])
                sdb = sd_a[:, i : i + 1, :].broadcast_to([P, 3, J])
                ncdb = ncd_a[:, i : i + 1, :].broadcast_to([P, 3, J])
                sc_msa = cst[:][:, k3 : k3 + 1]
                sc_ca = cst[:][:, 3 + k3 : 4 + k3]
                sc_bond = cst[:][:, (6 if i % 2 == 0 else 9) + k3 :
                                  (7 if i % 2 == 0 else 10) + k3]

                fc = scratch.tile([P, 3, J], DT, tag="fc", name=f"fc{i}")
                eng("fc").mul(fc[:], g1[:], ca)
                fs = scratch.tile([P, 3, J], DT, tag="fs", name=f"fs{i}")
                eng("fs").mul(fs[:], g1[:], sa)

                t3 = scratch.tile([P, 3, J], DT, tag="t3", name=f"t3_{i}")
                eng("t3").tensor_mul(t3[:], g2h[:], sdb)
                t1 = scratch.tile([P, 3, J], DT, tag="t1", name=f"t1_{i}")
                eng("t1").tensor_mul(t1[:], g2h[:], cdb)
                t2 = scratch.tile([P, 3, J], DT, tag="t2", name=f"t2_{i}")
                eng("t2").tensor_mul(t2[:], m3[:], sdb)
                t4 = scratch.tile([P, 3, J], DT, tag="t4", name=f"t4_{i}")
                eng("t4").tensor_mul(t4[:], m3[:], ncdb)
                hh = scratch.tile([P, 3, J], DT, tag="hh", name=f"hh{i}")
                eng("hh").tensor_add(hh[:], t1[:], t2[:])

                g1n = statep.tile([P, 3, J], DT, tag="g1", name=f"g1_{i}")
                eng("g1n").scalar_tensor_tensor(
                    g1n[:], hh[:], sc_msa, fc[:], AOP.mult, AOP.add)
                g2n = statep.tile([P, 3, J], DT, tag="g2", name=f"g2_{i}")
                eng("g2n").scalar_tensor_tensor(
                    g2n[:], hh[:], sc_ca, fs[:], AOP.mult, AOP.add)
                m3n = statep.tile([P, 3, J], DT, tag="m3", name=f"m3_{i}")
                eng("m3n").tensor_add(m3n[:], t3[:], t4[:])

                slot = i % (2 * TP)
                pn_ap = pblock[:][:, 3 * J * slot : 3 * J * (slot + 1)].rearrange(
                    "p (c j) -> p c j", c=3)
                eng("pn").scalar_tensor_tensor(
                    pn_ap, g1n[:], sc_bond, p_prev_ap, AOP.mult, AOP.add)
                p_prev_ap = pn_ap
                g1, g2h, m3 = g1n, g2n, m3n

                # pace precompute: up to 1 sub-op per atom
                if pending:
                    pending.pop(0)()
                elif next_pair * CH < N:
                    pending = pair_ops(next_pair)
                    next_pair += 2
                    pending.pop(0)()

                # transpose piece each TP atoms: atoms [i-TP+1 .. i]
                # dims [P, j, x]: contiguous 36B writes, gathered reads
                if i % TP == TP - 1:
                    pc = i // TP
                    rsl = slice(3 * TP * (pc % 2), 3 * TP * (pc % 2 + 1))
                    xsl = slice(3 * TP * pc, 3 * TP * (pc + 1))
                    src = pblock[:].rearrange(
                        "p (x j) -> p j x", j=J)[:, :, rsl]
                    dst = stag[:].rearrange(
                        "p (j x) -> p j x", x=3 * N)[:, :, xsl]
                    e = eng("tp0") if pc % 2 == 0 else eng("tp1")
                    if e is nc.scalar:
                        e.copy(dst, src)
                    else:
                        e.tensor_copy(dst, src)

            nc.sync.dma_start(out=out_flat, in_=stag[:])
    return nc


_NC_CACHE: dict = {}


def _get_nc():
    if "nc" not in _NC_CACHE:
        nc = bacc.Bacc("TRN2", target_bir_lowering=False, debug=False)
        _emit(nc)
        nc.compile()
        _NC_CACHE["nc"] = nc
    return _NC_CACHE["nc"]


def run_sharded(angles: np.ndarray, prev_three: np.ndarray, **kw):
    angles = np.ascontiguousarray(angles).astype(NPDT)
    prev_three = np.ascontiguousarray(prev_three, np.float32)
    assert angles.shape == (B_FULL, 2 * N) and prev_three.shape == (B_FULL, 3, 3)
    in_maps = [
        {
            "angles": angles[i * BS : (i + 1) * BS],
            "prev_three": prev_three[i * BS : (i + 1) * BS],
        }
        for i in range(N_CORES)
    ]
    return run_bass_kernel_spmd(_get_nc(), in_maps, core_ids=list(range(N_CORES)), **kw)


def kernel(angles: np.ndarray, prev_three: np.ndarray) -> np.ndarray:
    res = run_sharded(angles, prev_three)
    return np.concatenate(
        [r["out"].astype(np.float32) for r in res.results], axis=0)


# revision 27
# speedup vs baseline: 1.0738x; 1.0738x over previous
"""Trainium2 Bass kernel for DihedralToCartesian (NeRF-style dihedral->xyz chain).

Full-input contract: kernel(angles[65536,252], prev_three[65536,3,3]) -> [65536,126,3].
Batch sharded 8 ways (8192 rows/core = [128 partitions x 64 cols], pure data
parallelism).

Math: the reference renormalizes its frame every step, so the frame follows the
PURE-UNIT dihedral direction (c,s)/sqrt(s^2+c^2) exactly; the 1e-8 damping only
shrinks one step's displacement by |damped| (~5e-7 off except ~4 dataset rows,
worst once-off ~1e-3 rel).  We therefore treat each step as an exact rotation
(no invw/invg normalizer chain):

    h = cd*f2 - sd*f3; u = ca*f1 + sa*h; p += bond*u
    f1' = -u; f2' = sa*f1 - ca*h; f3' = sd*f2 + cd*f3

Sign-folded states g1 = (-1)^(i+1) f1, g2h = (-1)^i f2, m3 = -(-1)^i f3 make
every step sign-free except the bond constant:
    hh  = cd*g2h + sd*m3         g1' = ca*g1 - sa*hh     g2h' = sa*g1 + ca*hh
    m3' = sd*g2h - cd*m3         p' = p + (+/-bond)*g1'
which is 6 plain TT multiplies/adds + 3 STT + 2 ACT const-scales per atom.

Everything runs in fp16 (validated vs the fp32 reference: rel err 9.5e-3 vs the
2e-2 gate): DVE gets its 2x 16-bit mode, SBUF port pressure and DMA bytes halve.
The rsqrt normalization chain stays per-op-rounded fp16 but with fp32-safe
structure.  Planes are stored ATOM-MAJOR so all per-atom loop operands are
contiguous.  Precompute is paced one sub-op per 2 atoms in chunk PAIRS with
activation phases batched (Ln,Ln,Exp,Exp) so table loads stay ~8 total.
Positions accumulate into a dense atom-major pblock; paced transpose pieces
copy it to a j-major staging buffer which DMAs out in ONE 128-descriptor
transfer (64 contiguous DRAM rows per partition).
"""

import json
import os
import sys

import numpy as np

for _p in ("/opt/trn_rl_repo", os.path.expanduser("~/.axon_site/_ro/trn_rl_repo")):
    if os.path.isdir(_p) and _p not in sys.path:
        sys.path.insert(0, _p)

import concourse.bass as bass
import concourse.bacc as bacc
import concourse.mybir as mybir
import concourse.tile as tile
from concourse.bass_utils import run_bass_kernel_spmd

F32 = mybir.dt.float32
F16 = mybir.dt.float16
AOP = mybir.AluOpType
AF = mybir.ActivationFunctionType

N_CORES = 8
B_FULL = 65536
BS = B_FULL // N_CORES
N = 126
P = 128
J = BS // P            # 64
CH = 18                # atoms per precompute chunk
GRP = 2                # chunks per pacing group (phase-batched)
TP = 6                 # atoms per transpose piece (N % TP == 0)

_ALPHA = np.array([2.028, 2.124, 1.941], np.float64)
_BOND = np.array([1.329, 1.458, 1.523], np.float64)
_CA = np.cos(_ALPHA)
_SA = np.sin(_ALPHA)

# engine assignment, overridable: KERN_ENG='{"t3":"v",...}'
# NOTE: scalar_tensor_tensor does NOT compile on Pool ("g") here -- STT ops
# (g1n/g2n/pn) must stay on "v".
ENG = {
    "t1": "v", "t2": "v", "hh": "v", "g1n": "v", "g2n": "v", "m3n": "v",
    "t4": "v", "t3": "g", "pn": "v",
    "fc": "s", "fs": "s",
    "tp0": "s", "tp1": "g",  # transpose piece engine, alternating
    # chunk sub-ops
    "sT": "s", "cT": "s", "s2": "s", "c2": "s", "lg": "s", "rv": "s",
    "nrv": "s", "ss": "g", "cdm": "g", "sdm": "v", "ncdm": "v",
}
ENG.update(json.loads(os.environ.get("KERN_ENG", "{}")))
DT32 = os.environ.get("DT32", "0") == "1"
DT = F32 if DT32 else F16
NPDT = np.float32 if DT32 else np.float16


def _emit(nc: bass.Bass):
    angles = nc.dram_tensor("angles", [BS, 2 * N], DT, kind="ExternalInput").ap()
    prev = nc.dram_tensor("prev_three", [BS, 3, 3], F32, kind="ExternalInput").ap()
    out = nc.dram_tensor("out", [BS, N, 3], DT, kind="ExternalOutput").ap()

    ang_flat = angles.rearrange("(p j) c -> p (j c)", p=P)      # [128, 16128]
    prev_flat = prev.rearrange("(p j) r c -> p (j r c)", p=P)   # [128, 576]
    out_flat = out.rearrange("(p j) a c -> p (j a c)", p=P)     # [128, 24192]

    def eng(key):
        return {"v": nc.vector, "g": nc.gpsimd, "s": nc.scalar}[ENG[key]]

    with tile.TileContext(nc) as tc:
        with (
            tc.tile_pool(name="raw", bufs=1) as rawp,
            tc.tile_pool(name="planes", bufs=1) as planesp,
            tc.tile_pool(name="chunk", bufs=1) as chunkp,
            tc.tile_pool(name="state", bufs=2) as statep,
            tc.tile_pool(name="scratch", bufs=2) as scratch,
            tc.tile_pool(name="big", bufs=1) as bigp,
            tc.tile_pool(name="pv", bufs=1) as pvp,
        ):
            raw = rawp.tile([P, J * 2 * N], DT, tag="raw")       # j-major
            cdp = planesp.tile([P, N * J], DT, tag="cdp")        # atom-major
            sdp = planesp.tile([P, N * J], DT, tag="sdp")
            ncdp = planesp.tile([P, N * J], DT, tag="ncdp")      # -cd (m3n as ADD)
            # per-residue STT scalars as fp16 [P,1] APs (2x mode eligibility)
            cst = pvp.tile([P, 12], DT, tag="cst")
            for k in range(3):
                nc.vector.memset(cst[:][:, k : k + 1], float(-_SA[k]))
                nc.vector.memset(cst[:][:, 3 + k : 4 + k], float(_CA[k]))
                nc.vector.memset(cst[:][:, 6 + k : 7 + k], float(_BOND[k]))
                nc.vector.memset(cst[:][:, 9 + k : 10 + k], float(-_BOND[k]))
            # ring of 2*TP atoms of dense p; transpose pieces drain it
            pblock = bigp.tile([P, 2 * TP * 3 * J], DT, tag="pblock")
            stag = bigp.tile([P, J * N * 3], DT, tag="stag")      # j-major p
            pv = pvp.tile([P, J * 9], F32, tag="pv")
            pvt = pvp.tile([P, 9 * J], F32, tag="pvt")
            ln24 = pvp.tile([P, 1], F32, tag="ln24")
            nc.vector.memset(ln24[:], float(np.log(24.0)))

            nc.sync.dma_start(out=pv[:], in_=prev_flat)
            nc.sync.dma_start(out=raw[:], in_=ang_flat)

            raw_aj = raw[:].rearrange("p (j a) -> p a j", a=2 * N)  # strided view
            cd_a = cdp[:].rearrange("p (a j) -> p a j", j=J)
            sd_a = sdp[:].rearrange("p (a j) -> p a j", j=J)
            ncd_a = ncdp[:].rearrange("p (a j) -> p a j", j=J)
            pb_v = pblock[:].rearrange("p (a x) -> p a x", x=3 * J)  # per-atom dense
            stag_r = stag[:].rearrange("p (j x) -> p x j", x=3 * N)

            # ---------- precompute: one chunk pair, phase-batched --------
            def pair_ops(k0):
                """Thunks for chunks k0, k0+1 (atoms [CH*k0, CH*k0+2*CH))."""
                ops = []
                tiles = {}
                for ci, k in enumerate((k0, k0 + 1)):
                    if k * CH >= N:
                        continue
                    a0 = k * CH
                    sl = slice(a0, a0 + CH)
                    csl = slice(N + a0, N + a0 + CH)
                    sfx = "a" if ci == 0 else "b"
                    SH = [P, CH, J]
                    st = chunkp.tile(SH, DT, tag=f"sT{sfx}", name=f"sT{k}")
                    ct = chunkp.tile(SH, DT, tag=f"cT{sfx}", name=f"cT{k}")
                    s2 = chunkp.tile(SH, DT, tag=f"s2{sfx}", name=f"s2{k}")
                    c2 = chunkp.tile(SH, DT, tag=f"c2{sfx}", name=f"c2{k}")
                    ssq = chunkp.tile(SH, DT, tag=f"ss{sfx}", name=f"ss{k}")
                    lg = chunkp.tile(SH, DT, tag=f"s2{sfx}", name=f"lg{k}")
                    rv = chunkp.tile(SH, DT, tag=f"c2{sfx}", name=f"rv{k}")
                    nrv = chunkp.tile(SH, DT, tag=f"ss{sfx}", name=f"nrv{k}")
                    tiles[k] = (st, ct, s2, c2, ssq, lg, rv, nrv, sl, csl)
                # phase-interleaved order: all sT, all cT, ..., so Ln/Exp batch
                for phase in range(11):
                    for k in (k0, k0 + 1):
                        if k not in tiles:
                            continue
                        st, ct, s2, c2, ssq, lg, rv, nrv, sl, csl = tiles[k]
                        # fp16-safe rsqrt: (24s)^2+(24c)^2 keeps tiny dihedrals
                        # out of fp16-subnormal range; Exp bias ln(24) undoes
                        # the scale exactly: exp(-0.5*ln(576*ss)+ln24) = ss^-0.5
                        if phase == 0:
                            ops.append(lambda st=st, sl=sl:
                                       eng("sT").copy(st[:], raw_aj[:, sl, :]))
                        elif phase == 1:
                            ops.append(lambda ct=ct, csl=csl:
                                       eng("cT").copy(ct[:], raw_aj[:, csl, :]))
                        elif phase == 2:
                            ops.append(lambda s2=s2, st=st:
                                       eng("s2").activation(s2[:], st[:], AF.Square, 0.0, 24.0))
                        elif phase == 3:
                            ops.append(lambda c2=c2, ct=ct:
                                       eng("c2").activation(c2[:], ct[:], AF.Square, 0.0, 24.0))
                        elif phase == 4:
                            ops.append(lambda ssq=ssq, s2=s2, c2=c2:
                                       eng("ss").tensor_add(ssq[:], s2[:], c2[:]))
                        elif phase == 5:
                            ops.append(lambda lg=lg, ssq=ssq:
                                       eng("lg").activation(lg[:], ssq[:], AF.Ln))
                        elif phase == 6:
                            ops.append(lambda rv=rv, lg=lg:
                                       eng("rv").activation(
                                           rv[:], lg[:], AF.Exp, ln24[:], -0.5))
                        elif phase == 7:
                            ops.append(lambda ct=ct, rv=rv, sl=sl:
                                       eng("cdm").tensor_mul(cd_a[:, sl, :], ct[:], rv[:]))
                        elif phase == 8:
                            ops.append(lambda st=st, rv=rv, sl=sl:
                                       eng("sdm").tensor_mul(sd_a[:, sl, :], st[:], rv[:]))
                        elif phase == 9:
                            ops.append(lambda nrv=nrv, rv=rv:
                                       eng("nrv").mul(nrv[:], rv[:], -1.0))
                        else:
                            ops.append(lambda ct=ct, nrv=nrv, sl=sl:
                                       eng("ncdm").tensor_mul(ncd_a[:, sl, :], ct[:], nrv[:]))
                return ops

            # ---------- initial frame from prev_three (fp32) -------------
            nc.scalar.copy(
                pvt[:].rearrange("p (x j) -> p x j", x=9),
                pv[:].rearrange("p (j x) -> p x j", x=9),
            )
            pvt_r = pvt[:].rearrange("p (x j) -> p x j", x=9)
            a0_ap = pvt_r[:, 0:3, :]
            b0_ap = pvt_r[:, 3:6, :]
            c0_ap = pvt_r[:, 6:9, :]

            def rsqrt3(dst, src3, tag):
                sq = scratch.tile([P, 3, J], F32, tag="i_sq", name=f"sq_{tag}")
                nc.scalar.square(sq[:], src3[:])
                s1 = scratch.tile([P, 1, J], F32, tag="i_s1", name=f"s1_{tag}")
                nc.vector.tensor_add(s1[:], sq[:, 0:1, :], sq[:, 1:2, :])
                s2_ = scratch.tile([P, 1, J], F32, tag="i_s2", name=f"s2_{tag}")
                nc.vector.tensor_add(s2_[:], s1[:], sq[:, 2:3, :])
                lgi = scratch.tile([P, 1, J], F32, tag="i_lg", name=f"lg_{tag}")
                nc.scalar.activation(lgi[:], s2_[:], AF.Ln)
                nc.scalar.activation(dst[:], lgi[:], AF.Exp, 0.0, -0.5)

            def cross_into(dst, x, y, eps):
                # dst[c] = (x[c1]*y[c2] + eps) - x[c2]*y[c1]
                for c in range(3):
                    c1, c2 = (c + 1) % 3, (c + 2) % 3
                    m = scratch.tile([P, 1, J], F32, tag="i_cm", name=f"cm{c}_{dst.name}")
                    q = scratch.tile([P, 1, J], F32, tag="i_cq", name=f"cq{c}_{dst.name}")
                    nc.vector.tensor_mul(m[:], x[:, c1 : c1 + 1, :], y[:, c2 : c2 + 1, :])
                    nc.vector.tensor_mul(q[:], x[:, c2 : c2 + 1, :], y[:, c1 : c1 + 1, :])
                    nc.vector.scalar_tensor_tensor(
                        dst[:, c : c + 1, :], m[:], eps, q[:], AOP.add, AOP.subtract)

            vv = scratch.tile([P, 3, J], F32, tag="i_vv")
            nc.vector.scalar_tensor_tensor(
                vv[:], b0_ap, 1e-8, c0_ap, AOP.add, AOP.subtract)
            rv1 = scratch.tile([P, 1, J], F32, tag="i_rv")
            rsqrt3(rv1, vv, "f1")
            f1f = scratch.tile([P, 3, J], F32, tag="i_f1")
            nc.vector.tensor_mul(f1f[:], vv[:], rv1[:].broadcast_to([P, 3, J]))
            g1 = statep.tile([P, 3, J], DT, tag="g1", name="g1_init")
            nc.scalar.copy(g1[:], f1f[:])

            uu = scratch.tile([P, 3, J], F32, tag="i_uu")
            nc.vector.tensor_sub(uu[:], b0_ap, a0_ap)
            # m3_init = +f3 = normalize(cross(b-a, f1) + 1e-8)
            ww = scratch.tile([P, 3, J], F32, tag="i_ww", name="i_ww")
            cross_into(ww, uu, f1f, 1e-8)
            rv2 = scratch.tile([P, 1, J], F32, tag="i_rv", name="i_rv2")
            rsqrt3(rv2, ww, "f3")
            f3f = scratch.tile([P, 3, J], F32, tag="i_f3")
            nc.vector.tensor_mul(f3f[:], ww[:], rv2[:].broadcast_to([P, 3, J]))
            m3 = statep.tile([P, 3, J], DT, tag="m3", name="m3_init")
            nc.scalar.copy(m3[:], f3f[:])
            # g2h_init = -f2 = cross(f1, f3)
            g2f = scratch.tile([P, 3, J], F32, tag="i_g2", name="i_g2")
            cross_into(g2f, f1f, f3f, 0.0)
            g2h = statep.tile([P, 3, J], DT, tag="g2", name="g2_init")
            nc.scalar.copy(g2h[:], g2f[:])
            # p_init = c0 in DT
            p0 = statep.tile([P, 3, J], DT, tag="pd", name="p_init")
            nc.scalar.copy(p0[:], c0_ap)
            p_prev_ap = p0[:]

            # pre-emit first chunk pair; pace the rest at 1 op / 2 atoms
            for f in pair_ops(0):
                f()
            pending = pair_ops(2)
            next_pair = 4

            # ---------- main recurrence ---------------------------------
            for i in range(N):
                k3 = i % 3
                ca, sa = float(_CA[k3]), float(_SA[k3])
                sbond = float(_BOND[k3] * (1.0 if i % 2 == 0 else -1.0))

                cdb = cd_a[:, i : i + 1, :].broadcast_to([P, 3, J])
                sdb = sd_a[:, i : i + 1, :].broadcast_to([P, 3, J])
                ncdb = ncd_a[:, i : i + 1, :].broadcast_to([P, 3, J])
                sc_msa = cst[:][:, k3 : k3 + 1]
                sc_ca = cst[:][:, 3 + k3 : 4 + k3]
                sc_bond = cst[:][:, (6 if i % 2 == 0 else 9) + k3 :
                                  (7 if i % 2 == 0 else 10) + k3]

                fc = scratch.tile([P, 3, J], DT, tag="fc", name=f"fc{i}")
                eng("fc").mul(fc[:], g1[:], ca)
                fs = scratch.tile([P, 3, J], DT, tag="fs", name=f"fs{i}")
                eng("fs").mul(fs[:], g1[:], sa)

                t3 = scratch.tile([P, 3, J], DT, tag="t3", name=f"t3_{i}")
                eng("t3").tensor_mul(t3[:], g2h[:], sdb)
                t1 = scratch.tile([P, 3, J], DT, tag="t1", name=f"t1_{i}")
                eng("t1").tensor_mul(t1[:], g2h[:], cdb)
                t2 = scratch.tile([P, 3, J], DT, tag="t2", name=f"t2_{i}")
                eng("t2").tensor_mul(t2[:], m3[:], sdb)
                t4 = scratch.tile([P, 3, J], DT, tag="t4", name=f"t4_{i}")
                eng("t4").tensor_mul(t4[:], m3[:], ncdb)
                hh = scratch.tile([P, 3, J], DT, tag="hh", name=f"hh{i}")
                eng("hh").tensor_add(hh[:], t1[:], t2[:])

                g1n = statep.tile([P, 3, J], DT, tag="g1", name=f"g1_{i}")
                eng("g1n").scalar_tensor_tensor(
                    g1n[:], hh[:], -sa, fc[:], AOP.mult, AOP.add)
                g2n = statep.tile([P, 3, J], DT, tag="g2", name=f"g2_{i}")
                eng("g2n").scalar_tensor_tensor(
                    g2n[:], hh[:], ca, fs[:], AOP.mult, AOP.add)
                m3n = statep.tile([P, 3, J], DT, tag="m3", name=f"m3_{i}")
                eng("m3n").tensor_add(m3n[:], t3[:], t4[:])

                slot = i % (2 * TP)
                pn_ap = pblock[:][:, 3 * J * slot : 3 * J * (slot + 1)].rearrange(
                    "p (c j) -> p c j", c=3)
                eng("pn").scalar_tensor_tensor(
                    pn_ap, g1n[:], sbond, p_prev_ap, AOP.mult, AOP.add)
                p_prev_ap = pn_ap
                g1, g2h, m3 = g1n, g2n, m3n

                # pace precompute: up to 1 sub-op per atom
                if pending:
                    pending.pop(0)()
                elif next_pair * CH < N:
                    pending = pair_ops(next_pair)
                    next_pair += 2
                    pending.pop(0)()

                # transpose piece each TP atoms: atoms [i-TP+1 .. i]
                # dims [P, j, x]: contiguous 36B writes, gathered reads
                if i % TP == TP - 1:
                    pc = i // TP
                    rsl = slice(3 * TP * (pc % 2), 3 * TP * (pc % 2 + 1))
                    xsl = slice(3 * TP * pc, 3 * TP * (pc + 1))
                    src = pblock[:].rearrange(
                        "p (x j) -> p j x", j=J)[:, :, rsl]
                    dst = stag[:].rearrange(
                        "p (j x) -> p j x", x=3 * N)[:, :, xsl]
                    e = eng("tp0") if pc % 2 == 0 else eng("tp1")
                    if e is nc.scalar:
                        e.copy(dst, src)
                    else:
                        e.tensor_copy(dst, src)

            nc.sync.dma_start(out=out_flat, in_=stag[:])
    return nc


_NC_CACHE: dict = {}


def _get_nc():
    if "nc" not in _NC_CACHE:
        nc = bacc.Bacc("TRN2", target_bir_lowering=False, debug=False)
        _emit(nc)
        nc.compile()
        _NC_CACHE["nc"] = nc
    return _NC_CACHE["nc"]


def run_sharded(angles: np.ndarray, prev_three: np.ndarray, **kw):
    angles = np.ascontiguousarray(angles).astype(NPDT)
    prev_three = np.ascontiguousarray(prev_three, np.float32)
    assert angles.shape == (B_FULL, 2 * N) and prev_three.shape == (B_FULL, 3, 3)
    in_maps = [
        {
            "angles": angles[i * BS : (i + 1) * BS],
            "prev_three": prev_three[i * BS : (i + 1) * BS],
        }
        for i in range(N_CORES)
    ]
    return run_bass_kernel_spmd(_get_nc(), in_maps, core_ids=list(range(N_CORES)), **kw)


def kernel(angles: np.ndarray, prev_three: np.ndarray) -> np.ndarray:
    res = run_sharded(angles, prev_three)
    return np.concatenate(
        [r["out"].astype(np.float32) for r in res.results], axis=0)
